# revision 1
# baseline (speedup 1.0000x reference)
"""Trainium2 Bass kernel for a decoder layer (self-attn + cross-attn + MLP,
custom global norm), sharded over 8 NeuronCores as 4 samples x 2 seq halves.

Layout: activations kept transposed [D, S] (d on partitions); weights used
directly as matmul lhsT tiles [d_in, d_out]. Matmuls run in float32r (TF32).
Cross-attention K/V projections are hoisted over norm1's AllReduce, and the
MLP's W1 matmul runs on pre-norm z2 via linearity (pe = a*z2 + b).
"""
import sys
sys.path.insert(0, '/opt/trn_rl_repo')
import numpy as np

B, D, S, H, DH, DFF = 4, 1024, 1024, 16, 64, 4096
N_CORES = 8
NUDGE = 1e-7
NTOT = float(D * S)
RG = [[0, 1], [2, 3], [4, 5], [6, 7]]


def round_tf32(x):
    b = np.ascontiguousarray(x, dtype=np.float32).view(np.uint32)
    return ((b + 0x1000) & 0xFFFFE000).view(np.float32)


def _split_multi_waits(nc, mybir):
    """walrus codegen allows at most one sync-wait command per instruction;
    move extra waits onto same-engine NoOps inserted just before."""
    n = 0
    for f in nc.m.functions:
        for bb in f.blocks:
            new_insts = []
            for inst in bb.instructions:
                si = getattr(inst, "sync_info", None)
                eng = getattr(inst, "engine", None)
                if si is not None and si.on_wait and len(si.on_wait) > 1 \
                        and eng is not None:
                    waits = list(si.on_wait)
                    for i, w in enumerate(waits[:-1]):
                        nop = mybir.InstNoOp(
                            name=f"{inst.name}-wsplit{i}",
                            engine=eng,
                            sync_info=mybir.SyncInfo(on_wait=[w], on_update=[]),
                            bass_nofuse=True,
                        )
                        new_insts.append(nop)
                        n += 1
                    si.on_wait = [waits[-1]]
                new_insts.append(inst)
            bb.instructions[:] = new_insts
    return n


def build_program():
    import concourse.bass as bass
    import concourse.tile as tile
    from concourse import mybir

    FP32 = mybir.dt.float32
    FP32R = mybir.dt.float32r
    AF = mybir.ActivationFunctionType
    ALU = mybir.AluOpType
    AX = mybir.AxisListType

    nc = bass.Bass("TRN2", target_bir_lowering=False, debug=False,
                   num_devices=N_CORES)

    def din(name, shape, dt=FP32R):
        return nc.dram_tensor(name, shape, dt, kind="ExternalInput").ap()

    x_d = din("x", [D, S])
    xq_d = din("xq", [D, 512])
    emb_d = din("emb", [D, S])
    mask_d = din("mask", [S, 512])
    wq_s_d = din("wq_s", [8, 128, 8, 128]); wk_s_d = din("wk_s", [8, 128, 8, 128])
    wv_s_d = din("wv_s", [2, 128, 8, 512]); wo_s_d = din("wo_s", [8, 128, 8, 128])
    wq_c_d = din("wq_c", [8, 128, 8, 128]); wk_c_d = din("wk_c", [8, 128, 8, 128])
    wv_c_d = din("wv_c", [2, 128, 8, 512]); wo_c_d = din("wo_c", [8, 128, 8, 128])
    w1_d = din("w1", [32, 128, 8, 128]); w2_d = din("w2", [8, 128, 32, 128])
    b1_d = din("b1m", [128, 32], FP32)
    b2_d = din("b2m", [128, 8], FP32)
    w1s_d = din("w1s", [128, 32], FP32)     # colsums of W1
    ones64_d = din("ones64", [128, 64])
    ones2_d = din("ones2", [128, 2])
    out_d = nc.dram_tensor("out", [D, 512], FP32, kind="ExternalOutput").ap()

    def r3(ap, inner):
        return ap.rearrange("(t p) m -> p t m", p=128)

    marks = []
    nc._phase_marks = marks

    def mark(nm):
        marks.append((nm, int(nc.next_id())))

    with tile.TileContext(nc) as tc:
        import contextlib
        ctx = contextlib.ExitStack()
        with ctx:
            persist = ctx.enter_context(tc.tile_pool(name="persist", bufs=1))
            dram = ctx.enter_context(
                tc.tile_pool(name="dram", bufs=1, space="DRAM"))
            actp = ctx.enter_context(tc.tile_pool(name="actp", bufs=3))

            def act_tile(nm):
                return actp.tile([128, 8, 512], FP32R, tag="act", name=nm)

            xq_sb = persist.tile([128, 8, 512], FP32R)
            nc.sync.dma_start(out=xq_sb, in_=r3(xq_d, 512))
            ones64_sb = persist.tile([128, 64], FP32R)
            nc.sync.dma_start(out=ones64_sb, in_=ones64_d)
            ones2_sb = persist.tile([128, 2], FP32R)
            nc.sync.dma_start(out=ones2_sb, in_=ones2_d)
            b1_sb = persist.tile([128, 32], FP32)
            nc.sync.dma_start(out=b1_sb, in_=b1_d)
            b2_sb = persist.tile([128, 8], FP32)
            nc.sync.dma_start(out=b2_sb, in_=b2_d)
            w1s_sb = persist.tile([128, 32], FP32)
            nc.sync.dma_start(out=w1s_sb, in_=w1s_d)

            cc_in = [dram.tile([1, 2], FP32, name=f"cc_in{i}", tag=f"cci{i}")
                     for i in range(3)]
            cc_out = [dram.tile([1, 2], FP32, name=f"cc_out{i}", tag=f"cco{i}")
                      for i in range(3)]

            def norm_stats(z_sb, cc_idx, statp):
                """Local sums -> pairwise AllReduce -> rcol/nbias columns in
                statp's st tile. Returns (rcol, nbias) APs."""
                st = statp.tile([128, 8], FP32, tag="st", name=f"st{cc_idx}")
                sqp = tc.alloc_tile_pool(name=f"sq{cc_idx}", bufs=2)
                with tc.tile_pool(name=f"npsum{cc_idx}", bufs=1,
                                  space="PSUM") as npsum:
                    n1 = npsum.tile([2, 512], FP32, tag="n1")
                    n2 = npsum.tile([2, 512], FP32, tag="n2")
                    for di in range(8):
                        sq = sqp.tile([128, 512], FP32R, tag="sq")
                        nc.vector.tensor_mul(sq, z_sb[:, di, :],
                                             z_sb[:, di, :])
                        nc.tensor.matmul(n1, ones2_sb, z_sb[:, di, :],
                                         start=(di == 0), stop=(di == 7))
                        nc.tensor.matmul(n2, ones2_sb, sq,
                                         start=(di == 0), stop=(di == 7))
                    nc.vector.tensor_reduce(st[0:1, 6:7], n1[0:1, :],
                                            AX.X, ALU.add)
                    nc.vector.tensor_reduce(st[0:1, 7:8], n2[0:1, :],
                                            AX.X, ALU.add)
                sqp.release()
                nc.sync.dma_start(out=cc_in[cc_idx], in_=st[0:1, 6:8])
                nc.gpsimd.collective_compute(
                    "AllReduce", ALU.add, replica_groups=RG,
                    ins=[cc_in[cc_idx]], outs=[cc_out[cc_idx]])
                gs = st[:, 4:6]
                bco = cc_out[cc_idx]
                bcast = bass.AP(tensor=bco.tensor, offset=bco.offset,
                                ap=[[0, 128], [1, 2]])
                nc.sync.dma_start(out=gs, in_=bcast)
                s1, s2 = gs[:, 0:1], gs[:, 1:2]
                mean, tmp = st[:, 0:1], st[:, 1:2]
                rcol, nbias = st[:, 2:3], st[:, 3:4]
                nc.vector.tensor_scalar_mul(mean, s1, 1.0 / NTOT)
                nc.vector.tensor_mul(tmp, mean, s1)
                nc.vector.tensor_sub(tmp, s2, tmp)
                nc.scalar.sqrt(tmp, tmp)
                nc.vector.tensor_scalar_add(tmp, tmp, NUDGE)
                nc.vector.reciprocal(rcol, tmp)
                nc.vector.tensor_mul(nbias, mean, rcol)
                nc.vector.tensor_scalar_mul(nbias, nbias, -1.0)
                return rcol, nbias

            def norm_apply(z_sb, dst_sb, rcol, nbias):
                for di in range(8):
                    nc.vector.tensor_scalar(dst_sb[:, di, :], z_sb[:, di, :],
                                            rcol, nbias, ALU.mult, ALU.add)

            def kproj(dst_sb, src_sb, w_dram, wpool, ppool, nsblk):
                for do in range(8):
                    wblk = wpool.tile([128, 8, 128], FP32R, tag="kw")
                    nc.sync.dma_start(out=wblk, in_=w_dram[do])
                    for sb_ in range(nsblk):
                        ps = ppool.tile([128, 512], FP32, tag="pp")
                        for di in range(8):
                            nc.tensor.matmul(
                                ps, wblk[:, di, :],
                                src_sb[:, di, sb_ * 512:(sb_ + 1) * 512],
                                start=(di == 0), stop=(di == 7))
                        nc.scalar.copy(
                            dst_sb[:, do, sb_ * 512:(sb_ + 1) * 512], ps)

            def kv_projections(kv_loader, wk_dr, wv_dr, kvp, wpool, aph):
                """K^T ([d,s]) and V ([s,d]) from the kv source."""
                kt_sb = kvp.tile([128, 8, 1024], FP32R, tag="kt")
                v_sb = kvp.tile([128, 8, 1024], FP32R, tag="v")
                with (
                    tc.tile_pool(name=f"src{aph}", bufs=1) as srcp,
                    tc.tile_pool(name=f"wv{aph}", bufs=1) as wvp,
                    tc.tile_pool(name=f"pp{aph}", bufs=4,
                                 space="PSUM") as ppool,
                ):
                    src_sb = kv_loader(srcp)
                    kproj(kt_sb, src_sb, wk_dr, wpool, ppool, 2)
                    for dvb in range(2):
                        wvh = wvp.tile([128, 8, 512], FP32R, tag="wv")
                        nc.sync.dma_start(out=wvh, in_=wv_dr[dvb])
                        for st_ in range(8):
                            ps = ppool.tile([128, 512], FP32, tag="pp")
                            for di in range(8):
                                nc.tensor.matmul(
                                    ps,
                                    src_sb[:, di, st_ * 128:(st_ + 1) * 128],
                                    wvh[:, di, :],
                                    start=(di == 0), stop=(di == 7))
                            nc.vector.tensor_copy(
                                v_sb[:, st_, dvb * 512:(dvb + 1) * 512], ps)
                return kt_sb, v_sb

            def attn_rest(kt_sb, v_sb, q_src_sb, wq_dr, wo_dr, use_mask,
                          resid_sb, z_sb, aout_sb, kvp, wpool, aph):
                """Q proj, per-head attention, Wo, residual."""
                mark(f'attn{aph}_start')
                qt_sb = kvp.tile([128, 8, 512], FP32R, tag="qt")
                with tc.tile_pool(name=f"qp{aph}", bufs=4,
                                  space="PSUM") as ppool:
                    kproj(qt_sb, q_src_sb, wq_dr, wpool, ppool, 1)

                with (
                    tc.tile_pool(name=f"mk{aph}", bufs=1) as mkp,
                    tc.tile_pool(name=f"ep{aph}", bufs=5) as epool,
                    tc.tile_pool(name=f"dv{aph}", bufs=3) as dvp,
                    tc.tile_pool(name=f"scp{aph}", bufs=2,
                                 space="PSUM") as scp,
                    tc.tile_pool(name=f"avp{aph}", bufs=2,
                                 space="PSUM") as avp,
                ):
                    mask_sb = None
                    if use_mask:
                        mask_sb = mkp.tile([128, 8, 512], FP32R, tag="mask")
                        nc.sync.dma_start(out=mask_sb, in_=r3(mask_d, 512))
                    mark(f'attn{aph}_heads')
                    for h in range(H):
                        off = (h % 2) * 64
                        hp = h // 2
                        e_tiles = []
                        for tt in range(4):
                            sc = scp.tile([128, 2, 512], FP32, tag="sc")
                            for j in range(2):
                                kt = 2 * tt + j
                                nc.tensor.matmul(
                                    sc[:, j, :],
                                    kt_sb[off:off + 64, hp,
                                          kt * 128:(kt + 1) * 128],
                                    qt_sb[off:off + 64, hp, :],
                                    start=True, stop=True,
                                    tile_position=(off, 0))
                            e = epool.tile([128, 2, 512], FP32R, tag="e")
                            nc.scalar.activation(e, sc, AF.Exp, scale=0.125)
                            if mask_sb is not None:
                                nc.vector.tensor_mul(
                                    e, e, mask_sb[:, 2 * tt:2 * tt + 2, :])
                            e_tiles.append(e)
                        av = avp.tile([128, 512], FP32, tag="av")
                        dn = avp.tile([128, 512], FP32, tag="dn")
                        for kt in range(8):
                            rhs = e_tiles[kt // 2][:, kt % 2, :]
                            nc.tensor.matmul(
                                av[0:64, :],
                                v_sb[:, kt, h * 64:(h + 1) * 64], rhs,
                                start=(kt == 0), stop=(kt == 7))
                            nc.tensor.matmul(
                                dn[0:64, :], ones64_sb, rhs,
                                start=(kt == 0), stop=(kt == 7))
                        rec = dvp.tile([128, 512], FP32, tag="rec")
                        nc.vector.reciprocal(rec[0:64, :], dn[0:64, :])
                        if off == 0:
                            nc.vector.tensor_mul(aout_sb[0:64, hp, :],
                                                 av[0:64, :], rec[0:64, :])
                        else:
                            tmp = dvp.tile([128, 512], FP32R, tag="tmp")
                            nc.vector.tensor_mul(tmp[0:64, :], av[0:64, :],
                                                 rec[0:64, :])
                            nc.sync.dma_start(out=aout_sb[64:128, hp, :],
                                              in_=tmp[0:64, :])

                mark(f'attn{aph}_wo')
                with tc.tile_pool(name=f"wops{aph}", bufs=3,
                                  space="PSUM") as wops:
                    for do in range(8):
                        wblk = wpool.tile([128, 8, 128], FP32R, tag="kw")
                        nc.sync.dma_start(out=wblk, in_=wo_dr[do])
                        ps = wops.tile([128, 512], FP32, tag="wo")
                        for di in range(8):
                            nc.tensor.matmul(ps, wblk[:, di, :],
                                             aout_sb[:, di, :],
                                             start=(di == 0), stop=(di == 7))
                        nc.vector.tensor_add(z_sb[:, do, :], ps,
                                             resid_sb[:, do, :])

            # ================= self attention =================
            z1_sb = act_tile("z1")
            stat1 = tc.alloc_tile_pool(name="stat1", bufs=1)
            with tc.tile_pool(name="kvS", bufs=1) as kvS:

                def load_x(pool):
                    x_sb = pool.tile([128, 8, 1024], FP32R, tag="src")
                    nc.sync.dma_start(out=x_sb, in_=r3(x_d, 1024))
                    return x_sb

                with tc.tile_pool(name="wstrS", bufs=3) as wpoolS:
                    ktS, vS = kv_projections(load_x, wk_s_d, wv_s_d, kvS,
                                             wpoolS, "s")
                    mark('kvproj_s_done')
                    aoutS = act_tile("aoutS")
                    attn_rest(ktS, vS, xq_sb, wq_s_d, wo_s_d, True,
                              xq_sb, z1_sb, aoutS, kvS, wpoolS, "s")
            # norm1 stats: the AllReduce overlaps cross K/V projections
            rcol1, nbias1 = norm_stats(z1_sb, 0, stat1)
            mark('norm1_stats_done')

            # ============= cross attention =============
            stat2 = tc.alloc_tile_pool(name="stat2", bufs=1)
            with tc.tile_pool(name="kvC", bufs=1) as kvC:

                def load_emb(pool):
                    e_sb = pool.tile([128, 8, 1024], FP32R, tag="src")
                    nc.sync.dma_start(out=e_sb, in_=r3(emb_d, 1024))
                    return e_sb

                with tc.tile_pool(name="wstrC", bufs=3) as wpoolC:
                    ktC, vC = kv_projections(load_emb, wk_c_d, wv_c_d, kvC,
                                             wpoolC, "c")
                    mark('kvproj_c_done')
                    pa_sb = act_tile("pa")
                    norm_apply(z1_sb, pa_sb, rcol1, nbias1)
                    aoutC = act_tile("aoutC")
                    z2_sb = act_tile("z2")
                    attn_rest(ktC, vC, pa_sb, wq_c_d, wo_c_d, False,
                              pa_sb, z2_sb, aoutC, kvC, wpoolC, "c")
                # norm2 stats start here; W1 @ z2 overlaps the AllReduce
                rcol2, nbias2 = norm_stats(z2_sb, 1, stat2)
                mark('norm2_stats_done')

            # ================= MLP =================
            with (
                tc.tile_pool(name="mlp", bufs=1) as mlp,
                tc.tile_pool(name="w1str", bufs=3) as w1str,
                tc.tile_pool(name="w2str", bufs=2) as w2str,
            ):
                # M = W1.T @ z2 (pre-norm); then h1 = relu(a*M + b*w1s + b1)
                mark('mlp_w1')
                m_sb = mlp.tile([128, 32, 512], FP32R, tag="h1")
                with tc.tile_pool(name="m1ps", bufs=4, space="PSUM") as m1ps:
                    for f in range(32):
                        wblk = w1str.tile([128, 8, 128], FP32R, tag="w1")
                        nc.sync.dma_start(out=wblk, in_=w1_d[f])
                        ps = m1ps.tile([128, 512], FP32, tag="m1")
                        for di in range(8):
                            nc.tensor.matmul(ps, wblk[:, di, :],
                                             z2_sb[:, di, :],
                                             start=(di == 0), stop=(di == 7))
                        nc.vector.tensor_copy(m_sb[:, f, :], ps)
                # per-f bias: b*w1s + b1, then in-place relu(a*M + bias)
                biasf = mlp.tile([128, 32], FP32, tag="biasf")
                nc.vector.tensor_scalar(biasf, w1s_sb, nbias2, None, ALU.mult)
                nc.vector.tensor_add(biasf, biasf, b1_sb)
                pe_sb = act_tile("pe")
                norm_apply(z2_sb, pe_sb, rcol2, nbias2)
                h1_sb = m_sb
                for f in range(32):
                    nc.scalar.activation(h1_sb[:, f, :],
                                         m_sb[:, f, :].bitcast(FP32),
                                         AF.Relu, bias=biasf[:, f:f + 1],
                                         scale=rcol2)
                mark('mlp_w2')
                z3_sb = act_tile("z3")
                with tc.tile_pool(name="m2ps", bufs=3, space="PSUM") as m2ps:
                    for do in range(8):
                        w2blk = w2str.tile([128, 32, 128], FP32R, tag="w2")
                        nc.sync.dma_start(out=w2blk, in_=w2_d[do])
                        ps = m2ps.tile([128, 512], FP32, tag="m2")
                        for ff in range(32):
                            nc.tensor.matmul(ps, w2blk[:, ff, :],
                                             h1_sb[:, ff, :],
                                             start=(ff == 0), stop=(ff == 31))
                        nc.vector.scalar_tensor_tensor(
                            z3_sb[:, do, :], ps, b2_sb[:, do:do + 1],
                            pe_sb[:, do, :], ALU.add, ALU.add)
                mark('norm3')
                stat3 = tc.alloc_tile_pool(name="stat3", bufs=1)
                rcol3, nbias3 = norm_stats(z3_sb, 2, stat3)
                out_sb = mlp.tile([128, 8, 512], FP32, tag="h1")
                norm_apply(z3_sb, out_sb, rcol3, nbias3)
                nc.sync.dma_start(out=r3(out_d, 512), in_=out_sb)
                stat3.release()
            stat2.release()
            stat1.release()

    from concourse import mybir as _mb
    _split_multi_waits(nc, _mb)
    return nc


_CACHE = {}


def _get_program():
    if "nc" not in _CACHE:
        _CACHE["nc"] = build_program()
    return _CACHE["nc"]


def _blk(w, nblk, blk):
    """[K, N] -> [nblk, 128, K//128, blk] contiguous per-column-block tiles."""
    K = w.shape[0]
    return np.ascontiguousarray(
        w.reshape(K // 128, 128, nblk, blk).transpose(2, 1, 0, 3))


def _make_in_maps(inputs):
    w_shared = {}
    for k in ("Wq_s", "Wk_s", "Wo_s", "Wq_c", "Wk_c", "Wo_c"):
        w_shared[k.lower()] = _blk(round_tf32(inputs[k]), 8, 128)
    for k in ("Wv_s", "Wv_c"):
        w_shared[k.lower()] = _blk(round_tf32(inputs[k]), 2, 512)
    w_shared["w1"] = _blk(round_tf32(inputs["W1"]), 32, 128)
    w_shared["w2"] = _blk(round_tf32(inputs["W2"]), 8, 128)
    b1m = np.ascontiguousarray(
        np.asarray(inputs["b1"], np.float32).reshape(32, 128).T)
    b2m = np.ascontiguousarray(
        np.asarray(inputs["b2"], np.float32).reshape(8, 128).T)
    w1s = np.ascontiguousarray(
        round_tf32(inputs["W1"]).sum(axis=0, dtype=np.float64).astype(
            np.float32).reshape(32, 128).T)
    ones64 = np.ones((128, 64), np.float32)
    ones2 = np.ones((128, 2), np.float32)

    in_maps = []
    for c in range(N_CORES):
        b, h = c // 2, c % 2
        x_r = round_tf32(inputs["other_inputs"][b])
        emb_r = round_tf32(inputs["embedding"][b])
        qg = h * 512 + np.arange(512)
        mask = (np.arange(S)[:, None] <= qg[None, :]).astype(np.float32)
        m = {
            "x": x_r,
            "xq": np.ascontiguousarray(x_r[:, h * 512:(h + 1) * 512]),
            "emb": emb_r,
            "mask": mask,
            "b1m": b1m, "b2m": b2m, "w1s": w1s,
            "ones64": ones64, "ones2": ones2,
        }
        m.update(w_shared)
        in_maps.append(m)
    return in_maps


def run(inputs, trace=False):
    from concourse.bass_utils import run_bass_kernel_spmd
    nc = _get_program()
    in_maps = _make_in_maps(inputs)
    res = run_bass_kernel_spmd(nc, in_maps, list(range(N_CORES)), trace=trace)
    out = np.zeros((B, D, S), np.float32)
    for c in range(N_CORES):
        b, h = c // 2, c % 2
        out[b][:, h * 512:(h + 1) * 512] = res.results[c]["out"]
    return out, res


def kernel(**inputs):
    out, _ = run(inputs, trace=False)
    return out



# revision 8
# speedup vs baseline: 1.4098x; 1.4098x over previous
"""Trainium2 Bass kernel for a decoder layer (self-attn + cross-attn + MLP,
custom global norm), sharded over 8 NeuronCores as 4 samples x 2 q-sets.

Layout: activations [D, S] (d on partitions); weights as lhsT blocks.
Matmuls in bf16 (fp32 PSUM accumulate); residual stream kept fp32.
Causal skip via stride-2 q-tile assignment (core h owns q-tiles
{6,4,2,0} or {7,5,3,1}, descending) -> identical score-width table on
both cores; masks are per-core data. Softmax denominator comes free from
a ones-augmented V (row 64 of the AV psum); reciprocal is batched
[16,512] and broadcast via a DRAM-roundtrip stride-0 DMA. Norm stats are
fused into the z-producing ops via accum_out; cross-core reduction is a
2-float AllReduce per norm over core pairs.
"""
import sys
sys.path.insert(0, '/opt/trn_rl_repo')
import numpy as np
import ml_dtypes

BF16NP = ml_dtypes.bfloat16

B, D, S, H, DH, DFF = 4, 1024, 1024, 16, 64, 4096
N_CORES = 8
NUDGE = 1e-7
NTOT = float(D * S)
RG = [[0, 1], [2, 3], [4, 5], [6, 7]]
# uniform per-kt score width (q cols 0:n are valid, descending q-tiles)
KTN = [512, 512, 384, 384, 256, 256, 128, 128]
QTS0 = [6, 4, 2, 0]   # local q-tile order, core h==0
QTS1 = [7, 5, 3, 1]   # core h==1


def _split_multi_waits(nc, mybir):
    """walrus codegen allows at most one sync-wait command per instruction;
    move extra waits onto same-engine NoOps inserted just before."""
    n = 0
    for f in nc.m.functions:
        for bb in f.blocks:
            new_insts = []
            for inst in bb.instructions:
                si = getattr(inst, "sync_info", None)
                eng = getattr(inst, "engine", None)
                if si is not None and si.on_wait and len(si.on_wait) > 1 \
                        and eng is not None:
                    waits = list(si.on_wait)
                    for i, w in enumerate(waits[:-1]):
                        nop = mybir.InstNoOp(
                            name=f"{inst.name}-wsplit{i}",
                            engine=eng,
                            sync_info=mybir.SyncInfo(on_wait=[w], on_update=[]),
                            bass_nofuse=True,
                        )
                        new_insts.append(nop)
                        n += 1
                    si.on_wait = [waits[-1]]
                new_insts.append(inst)
            bb.instructions[:] = new_insts
    return n


def build_program():
    import concourse.bass as bass
    import concourse.tile as tile
    from concourse import mybir

    FP32 = mybir.dt.float32
    BF = mybir.dt.bfloat16
    AF = mybir.ActivationFunctionType
    ALU = mybir.AluOpType
    AX = mybir.AxisListType

    nc = bass.Bass("TRN2", target_bir_lowering=False, debug=False,
                   num_devices=N_CORES)

    def din(name, shape, dt=BF):
        return nc.dram_tensor(name, shape, dt, kind="ExternalInput").ap()

    x_d = din("x", [128, 8, 1024])
    xq_d = din("xq", [128, 8, 512])
    xqf_d = din("xqf", [128, 8, 512], FP32)
    emb_d = din("emb", [128, 8, 1024])
    msk_d = din("msk", [128, 8, 128])
    wq_s_d = din("wq_s", [8, 128, 8, 128]); wk_s_d = din("wk_s", [8, 128, 8, 128])
    wv_s_d = din("wv_s", [2, 128, 8, 512]); wo_s_d = din("wo_s", [8, 128, 8, 128])
    wq_c_d = din("wq_c", [8, 128, 8, 128]); wk_c_d = din("wk_c", [8, 128, 8, 128])
    wv_c_d = din("wv_c", [2, 128, 8, 512]); wo_c_d = din("wo_c", [8, 128, 8, 128])
    w1_d = din("w1", [32, 128, 8, 128]); w2_d = din("w2", [8, 128, 32, 128])
    b1_d = din("b1m", [128, 32], FP32)
    b2_d = din("b2m", [128, 8], FP32)
    w1s_d = din("w1s", [128, 32], FP32)
    onesf_d = din("onesf", [128, 1], FP32)
    out_d = nc.dram_tensor("out", [128, 8, 512], FP32,
                           kind="ExternalOutput").ap()

    with tile.TileContext(nc) as tc:
        import contextlib
        ctx = contextlib.ExitStack()
        with ctx:
            persist = ctx.enter_context(tc.tile_pool(name="persist", bufs=1))
            dram = ctx.enter_context(
                tc.tile_pool(name="dram", bufs=1, space="DRAM"))

            xq_sb = persist.tile([128, 8, 512], BF)
            nc.sync.dma_start(out=xq_sb, in_=xq_d)
            xqf_sb = persist.tile([128, 8, 512], FP32)
            nc.sync.dma_start(out=xqf_sb, in_=xqf_d)
            msk_sb = persist.tile([128, 8, 128], BF)
            nc.sync.dma_start(out=msk_sb, in_=msk_d)
            b1_sb = persist.tile([128, 32], FP32)
            nc.sync.dma_start(out=b1_sb, in_=b1_d)
            b2_sb = persist.tile([128, 8], FP32)
            nc.sync.dma_start(out=b2_sb, in_=b2_d)
            w1s_sb = persist.tile([128, 32], FP32)
            nc.sync.dma_start(out=w1s_sb, in_=w1s_d)
            onesf_sb = persist.tile([128, 1], FP32)
            nc.sync.dma_start(out=onesf_sb, in_=onesf_d)

            cc_in = [dram.tile([1, 2], FP32, name=f"cc_in{i}", tag=f"cci{i}")
                     for i in range(3)]
            cc_out = [dram.tile([1, 2], FP32, name=f"cc_out{i}", tag=f"cco{i}")
                      for i in range(3)]
            dnr_dr = [dram.tile([16, 512], FP32, name=f"dnr{i}", tag=f"dnr{i}")
                      for i in range(2)]

            def stats_finish(cc_idx, cols_sb, statp):
                """cols [128,16] (z sums in 0:8, sq sums in 8:16) ->
                AllReduce -> (rcol, nbias) columns."""
                st = statp.tile([128, 8], FP32, tag=f"st{cc_idx}",
                                name=f"st{cc_idx}")
                with tc.tile_pool(name=f"sps{cc_idx}", bufs=1,
                                  space="PSUM") as sps:
                    ps = sps.tile([1, 16], FP32, tag="sp")
                    nc.tensor.matmul(ps, onesf_sb, cols_sb,
                                     start=True, stop=True)
                    # sum the two groups of 8 into adjacent scalars
                    nc.vector.tensor_reduce(st[0:1, 6:7], ps[0:1, 0:8],
                                            AX.X, ALU.add)
                    nc.vector.tensor_reduce(st[0:1, 7:8], ps[0:1, 8:16],
                                            AX.X, ALU.add)
                nc.sync.dma_start(out=cc_in[cc_idx], in_=st[0:1, 6:8])
                nc.gpsimd.collective_compute(
                    "AllReduce", ALU.add, replica_groups=RG,
                    ins=[cc_in[cc_idx]], outs=[cc_out[cc_idx]])
                gs = st[:, 4:6]
                bco = cc_out[cc_idx]
                bcast = bass.AP(tensor=bco.tensor, offset=bco.offset,
                                ap=[[0, 128], [1, 2]])
                nc.sync.dma_start(out=gs, in_=bcast)
                s1, s2 = gs[:, 0:1], gs[:, 1:2]
                mean, tmp = st[:, 0:1], st[:, 1:2]
                rcol, nbias = st[:, 2:3], st[:, 3:4]
                nc.vector.tensor_scalar_mul(mean, s1, 1.0 / NTOT)
                nc.vector.tensor_mul(tmp, mean, s1)
                nc.vector.tensor_sub(tmp, s2, tmp)
                nc.scalar.sqrt(tmp, tmp)
                nc.vector.tensor_scalar_add(tmp, tmp, NUDGE)
                nc.vector.reciprocal(rcol, tmp)
                nc.vector.tensor_mul(nbias, mean, rcol)
                nc.vector.tensor_scalar_mul(nbias, nbias, -1.0)
                return rcol, nbias

            kp_ctr = [0]

            def kproj(dst_sb, src_sb, w_dram, wpool, nsblk, copy_eng):
                """dst[do-block] = W^T src, LDW reused across s-blocks."""
                kp_ctr[0] += 1
                with tc.tile_pool(name=f"kp{kp_ctr[0]}", bufs=4,
                                  space="PSUM") as ppool:
                    for do in range(8):
                        wblk = wpool.tile([128, 8, 128], BF, tag="kw")
                        nc.sync.dma_start(out=wblk, in_=w_dram[do])
                        pss = [ppool.tile([128, 512], FP32, tag="pp",
                                          name=f"pp{do}_{i}")
                               for i in range(nsblk)]
                        for di in range(8):
                            for sb_ in range(nsblk):
                                nc.tensor.matmul(
                                    pss[sb_], wblk[:, di, :],
                                    src_sb[:, di, sb_ * 512:(sb_ + 1) * 512],
                                    start=(di == 0), stop=(di == 7))
                        for sb_ in range(nsblk):
                            eng = copy_eng(do, sb_)
                            if eng == "s":
                                nc.scalar.copy(
                                    dst_sb[:, do, sb_ * 512:(sb_ + 1) * 512],
                                    pss[sb_])
                            else:
                                nc.vector.tensor_copy(
                                    dst_sb[:, do, sb_ * 512:(sb_ + 1) * 512],
                                    pss[sb_])

            def vproj(v_sb, src_sb, wv_dram, aph):
                """v_sb [128, 8st, 16h, 65]; col 64 = ones (dn trick)."""
                with (
                    tc.tile_pool(name=f"wv{aph}", bufs=1) as wvp,
                    tc.tile_pool(name=f"vp{aph}", bufs=4,
                                 space="PSUM") as ppool,
                ):
                    wvh = [wvp.tile([128, 8, 512], BF, tag=f"wv{dvb}",
                                    name=f"wvh{dvb}")
                           for dvb in range(2)]
                    for dvb in range(2):
                        nc.sync.dma_start(out=wvh[dvb], in_=wv_dram[dvb])
                    nc.vector.memset(v_sb[:, :, :, 64:65], 1.0)
                    for st_ in range(8):
                        pss = [ppool.tile([128, 512], FP32, tag="vp",
                                          name=f"vp{st_}_{i}")
                               for i in range(2)]
                        for di in range(8):
                            for dvb in range(2):
                                nc.tensor.matmul(
                                    pss[dvb],
                                    src_sb[:, di, st_ * 128:(st_ + 1) * 128],
                                    wvh[dvb][:, di, :],
                                    start=(di == 0), stop=(di == 7))
                        for dvb in range(2):
                            src = pss[dvb].rearrange("p (h m) -> p h m", h=8)
                            nc.vector.tensor_copy(
                                v_sb[:, st_, dvb * 8:(dvb + 1) * 8, 0:64],
                                src)

            def attention(kt_sb, v_sb, qt_sb, wo_dr, use_mask, resid_sb,
                          z_sb, zcols, wpool, dn_idx, aph):
                """Per-head scores/exp/AV with causal width table (self) or
                full table (cross); batched softmax normalize; Wo; residual
                z = Wo-out + resid with fused stat accumulation."""
                ktn = KTN if use_mask else [512] * 8
                aoutf = None
                with (
                    tc.tile_pool(name=f"ao{aph}", bufs=1) as aop,
                    tc.tile_pool(name=f"ep{aph}", bufs=6) as epool,
                    tc.tile_pool(name=f"dn{aph}", bufs=1) as dnpool,
                    tc.tile_pool(name=f"dnb{aph}", bufs=4) as dnbp,
                ):
                    aoutf = aop.tile([128, 8, 512], FP32, tag="aof")
                    aoutb = aop.tile([128, 8, 512], BF, tag="aob")
                    dnp = dnpool.tile([16, 512], FP32, tag="dn")
                    hctx = __import__("contextlib").ExitStack()
                    scp = hctx.enter_context(
                        tc.tile_pool(name=f"sc{aph}", bufs=2, space="PSUM"))
                    avp = hctx.enter_context(
                        tc.tile_pool(name=f"av{aph}", bufs=2, space="PSUM"))
                    for h in range(H):
                        off = (h % 2) * 64
                        hp = h // 2
                        e_tiles = []
                        for tt in range(4):
                            n0, n1 = ktn[2 * tt], ktn[2 * tt + 1]
                            sc = scp.tile([128, 2, 512], FP32, tag="sc")
                            for j, n in ((0, n0), (1, n1)):
                                kt = 2 * tt + j
                                nc.tensor.matmul(
                                    sc[:, j, 0:n],
                                    kt_sb[off:off + 64, hp,
                                          kt * 128:(kt + 1) * 128],
                                    qt_sb[off:off + 64, hp, 0:n],
                                    start=True, stop=True,
                                    tile_position=(off, 0))
                            e = epool.tile([128, 2, 512], BF, tag="e")
                            nc.scalar.activation(e[:, :, 0:n0],
                                                 sc[:, :, 0:n0],
                                                 AF.Exp, scale=0.125)
                            if use_mask:
                                w0 = n0 - 128
                                nc.vector.tensor_mul(
                                    e[:, :, w0:n0], e[:, :, w0:n0],
                                    msk_sb[:, 2 * tt:2 * tt + 2, :])
                            e_tiles.append(e)
                        av = avp.tile([128, 512], FP32, tag="av")
                        for kt in range(8):
                            n = ktn[kt]
                            rhs = e_tiles[kt // 2][:, kt % 2, 0:n]
                            nc.tensor.matmul(
                                av[0:65, 0:n],
                                v_sb[:, kt, h, :], rhs,
                                start=(kt == 0), stop=(kt == 7))
                        stage = dnbp.tile([65, 512], FP32, tag="stg",
                                          name=f"stg{aph}")
                        if h % 2 == 0:
                            nc.vector.tensor_copy(stage, av[0:65, :])
                        else:
                            nc.scalar.copy(stage, av[0:65, :])
                        nc.sync.dma_start(out=dnp[h:h + 1, :],
                                          in_=stage[64:65, :])
                        nc.sync.dma_start(out=aoutf[off:off + 64, hp, :],
                                          in_=stage[0:64, :])
                    hctx.close()
                    nc.vector.reciprocal(dnp, dnp)
                    nc.sync.dma_start(out=dnr_dr[dn_idx], in_=dnp)
                    for hp in range(8):
                        dnb = dnbp.tile([128, 512], FP32, tag="dnb")
                        for par in range(2):
                            src = dnr_dr[dn_idx][2 * hp + par]
                            bc = bass.AP(tensor=src.tensor, offset=src.offset,
                                         ap=[[0, 64], [1, 512]])
                            nc.sync.dma_start(
                                out=dnb[par * 64:(par + 1) * 64, :], in_=bc)
                        nc.vector.tensor_mul(aoutb[:, hp, :], aoutf[:, hp, :],
                                             dnb)
                    with tc.tile_pool(name=f"wops{aph}", bufs=3,
                                      space="PSUM") as wops:
                        for do in range(8):
                            wblk = wpool.tile([128, 8, 128], BF, tag="kw")
                            nc.sync.dma_start(out=wblk, in_=wo_dr[do])
                            ps = wops.tile([128, 512], FP32, tag="wo")
                            for di in range(8):
                                nc.tensor.matmul(ps, wblk[:, di, :],
                                                 aoutb[:, di, :],
                                                 start=(di == 0),
                                                 stop=(di == 7))
                            nc.vector.scalar_tensor_tensor(
                                z_sb[:, do, :], ps, 1.0, resid_sb[:, do, :],
                                ALU.mult, ALU.add,
                                accum_out=zcols[:, do:do + 1])
                            nc.scalar.activation(
                                sq_sc[:, do % 2, :], z_sb[:, do, :],
                                AF.Square, accum_out=zcols[:, 8 + do:9 + do])

            # scratch for Square outputs (values unused, accum_out is used)
            sq_sc = persist.tile([128, 2, 512], FP32)

            # ================= self attention =================
            statp = ctx.enter_context(tc.tile_pool(name="stats", bufs=1))
            colsp = ctx.enter_context(tc.tile_pool(name="cols", bufs=1))
            z1_sb = persist.tile([128, 8, 512], FP32)
            zcols1 = colsp.tile([128, 16], FP32, tag="zc1", name="zc1")
            with tc.tile_pool(name="kvS", bufs=1) as kvS, \
                 tc.tile_pool(name="srcS", bufs=1) as srcS:
                x_sb = srcS.tile([128, 8, 1024], BF, tag="x")
                nc.sync.dma_start(out=x_sb, in_=x_d)
                ktS = kvS.tile([128, 8, 1024], BF, tag="kt")
                vS = kvS.tile([128, 8, 16, 65], BF, tag="v")
                qtS = kvS.tile([128, 8, 512], BF, tag="qt")
                with tc.tile_pool(name="wstrS", bufs=3) as wpoolS:
                    kproj(ktS, x_sb, wk_s_d, wpoolS, 2,
                          lambda do, sb_: "s" if sb_ == 0 else "v")
                    vproj(vS, x_sb, wv_s_d, "s")
                    kproj(qtS, xq_sb, wq_s_d, wpoolS, 1, lambda do, sb_: "s")
                    attention(ktS, vS, qtS, wo_s_d, True, xqf_sb,
                              z1_sb, zcols1, wpoolS, 0, "s")
            rc1, nb1 = stats_finish(0, zcols1, statp)

            # ============= cross attention =============
            z2_sb = persist.tile([128, 8, 512], FP32)
            zcols2 = colsp.tile([128, 16], FP32, tag="zc2", name="zc2")
            pa_sb = persist.tile([128, 8, 512], BF)
            paf_sb = persist.tile([128, 8, 512], FP32)
            with tc.tile_pool(name="kvC", bufs=1) as kvC, \
                 tc.tile_pool(name="srcC", bufs=1) as srcC:
                e_sb = srcC.tile([128, 8, 1024], BF, tag="emb")
                nc.sync.dma_start(out=e_sb, in_=emb_d)
                ktC = kvC.tile([128, 8, 1024], BF, tag="kt")
                vC = kvC.tile([128, 8, 16, 65], BF, tag="v")
                qtC = kvC.tile([128, 8, 512], BF, tag="qt")
                with tc.tile_pool(name="wstrC", bufs=3) as wpoolC:
                    # K/V of emb overlap the norm1 AllReduce
                    kproj(ktC, e_sb, wk_c_d, wpoolC, 2,
                          lambda do, sb_: "s" if sb_ == 0 else "v")
                    vproj(vC, e_sb, wv_c_d, "c")
                    for di in range(8):
                        nc.scalar.activation(pa_sb[:, di, :], z1_sb[:, di, :],
                                             AF.Identity, bias=nb1, scale=rc1)
                        nc.vector.tensor_scalar(paf_sb[:, di, :],
                                                z1_sb[:, di, :], rc1, nb1,
                                                ALU.mult, ALU.add)
                    kproj(qtC, pa_sb, wq_c_d, wpoolC, 1, lambda do, sb_: "s")
                    attention(ktC, vC, qtC, wo_c_d, False, paf_sb,
                              z2_sb, zcols2, wpoolC, 1, "c")
            rc2, nb2 = stats_finish(1, zcols2, statp)

            # ================= MLP =================
            zcols3 = colsp.tile([128, 16], FP32, tag="zc3", name="zc3")
            with (
                tc.tile_pool(name="mlp", bufs=1) as mlp,
                tc.tile_pool(name="w1str", bufs=3) as w1str,
                tc.tile_pool(name="w2str", bufs=2) as w2str,
            ):
                # M = W1^T z2 (pre-norm; overlaps the norm2 AllReduce)
                z2b_sb = mlp.tile([128, 8, 512], BF, tag="z2b")
                for di in range(8):
                    nc.scalar.copy(z2b_sb[:, di, :], z2_sb[:, di, :])
                m_sb = mlp.tile([128, 32, 512], BF, tag="m")
                with tc.tile_pool(name="m1ps", bufs=4, space="PSUM") as m1ps:
                    for f in range(32):
                        wblk = w1str.tile([128, 8, 128], BF, tag="w1")
                        nc.sync.dma_start(out=wblk, in_=w1_d[f])
                        ps = m1ps.tile([128, 512], FP32, tag="m1")
                        for di in range(8):
                            nc.tensor.matmul(ps, wblk[:, di, :],
                                             z2b_sb[:, di, :],
                                             start=(di == 0), stop=(di == 7))
                        nc.vector.tensor_copy(m_sb[:, f, :], ps)
                # h1 = relu(rc2*M + (nb2*w1s + b1)); pe = rc2*z2 + nb2
                biasf = mlp.tile([128, 32], FP32, tag="biasf")
                nc.vector.tensor_scalar(biasf, w1s_sb, nb2, None, ALU.mult)
                nc.vector.tensor_add(biasf, biasf, b1_sb)
                pe_sb = mlp.tile([128, 8, 512], FP32, tag="pe")
                for di in range(8):
                    nc.vector.tensor_scalar(pe_sb[:, di, :], z2_sb[:, di, :],
                                            rc2, nb2, ALU.mult, ALU.add)
                h1_sb = m_sb
                for f in range(32):
                    nc.scalar.activation(h1_sb[:, f, :], m_sb[:, f, :],
                                         AF.Relu, bias=biasf[:, f:f + 1],
                                         scale=rc2)
                z3_sb = mlp.tile([128, 8, 512], FP32, tag="z3")
                with tc.tile_pool(name="m2ps", bufs=3, space="PSUM") as m2ps:
                    for do in range(8):
                        w2blk = w2str.tile([128, 32, 128], BF, tag="w2")
                        nc.sync.dma_start(out=w2blk, in_=w2_d[do])
                        ps = m2ps.tile([128, 512], FP32, tag="m2")
                        for ff in range(32):
                            nc.tensor.matmul(ps, w2blk[:, ff, :],
                                             h1_sb[:, ff, :],
                                             start=(ff == 0), stop=(ff == 31))
                        nc.vector.scalar_tensor_tensor(
                            z3_sb[:, do, :], ps, b2_sb[:, do:do + 1],
                            pe_sb[:, do, :], ALU.add, ALU.add,
                            accum_out=zcols3[:, do:do + 1])
                        nc.scalar.activation(
                            sq_sc[:, do % 2, :], z3_sb[:, do, :],
                            AF.Square, accum_out=zcols3[:, 8 + do:9 + do])
                rc3, nb3 = stats_finish(2, zcols3, statp)
                with tc.tile_pool(name="outp", bufs=2) as outp:
                    for do in range(8):
                        ot = outp.tile([128, 512], FP32, tag="ot")
                        nc.scalar.activation(ot, z3_sb[:, do, :],
                                             AF.Identity, bias=nb3, scale=rc3)
                        nc.sync.dma_start(out=out_d[:, do, :], in_=ot)

    from concourse import mybir as _mb
    _split_multi_waits(nc, _mb)
    return nc


_CACHE = {}


def _get_program():
    if "nc" not in _CACHE:
        _CACHE["nc"] = build_program()
    return _CACHE["nc"]


def _bf(x):
    return np.ascontiguousarray(np.asarray(x, np.float32)).astype(BF16NP)


def _blk(w, nblk, blk):
    """[K, N] -> [nblk, 128, K//128, blk] contiguous per-column-block."""
    K = w.shape[0]
    return np.ascontiguousarray(
        _bf(w).reshape(K // 128, 128, nblk, blk).transpose(2, 1, 0, 3))


def _dmaj(a):
    """[D, S] -> [128, 8, S] with d = t*128 + p."""
    return np.ascontiguousarray(
        np.asarray(a).reshape(8, 128, a.shape[1]).transpose(1, 0, 2))


def _make_in_maps(inputs):
    w_shared = {}
    for k in ("Wq_s", "Wk_s", "Wo_s", "Wq_c", "Wk_c", "Wo_c"):
        w_shared[k.lower()] = _blk(inputs[k], 8, 128)
    for k in ("Wv_s", "Wv_c"):
        w_shared[k.lower()] = _blk(inputs[k], 2, 512)
    w_shared["w1"] = _blk(inputs["W1"], 32, 128)
    w_shared["w2"] = _blk(inputs["W2"], 8, 128)
    b1m = np.ascontiguousarray(
        np.asarray(inputs["b1"], np.float32).reshape(32, 128).T)
    b2m = np.ascontiguousarray(
        np.asarray(inputs["b2"], np.float32).reshape(8, 128).T)
    w1s = np.ascontiguousarray(
        _bf(inputs["W1"]).astype(np.float64).sum(axis=0).astype(
            np.float32).reshape(32, 128).T)
    onesf = np.ones((128, 1), np.float32)
    tri = (np.arange(128)[:, None] <= np.arange(128)[None, :])

    in_maps = []
    for c in range(N_CORES):
        b, hh = c // 2, c % 2
        qts = QTS0 if hh == 0 else QTS1
        x_f = np.asarray(inputs["other_inputs"][b], np.float32)
        xq_f = np.concatenate([x_f[:, qt * 128:(qt + 1) * 128] for qt in qts],
                              axis=1)
        msk = np.zeros((128, 8, 128), np.float32)
        for kt in range(8):
            if hh == 0:
                msk[:, kt] = tri if kt % 2 == 0 else 0.0
            else:
                msk[:, kt] = 1.0 if kt % 2 == 0 else tri
        m = {
            "x": _bf(_dmaj(x_f)),
            "xq": _bf(_dmaj(xq_f)),
            "xqf": _dmaj(xq_f).astype(np.float32),
            "emb": _bf(_dmaj(np.asarray(inputs["embedding"][b], np.float32))),
            "msk": _bf(msk),
            "b1m": b1m, "b2m": b2m, "w1s": w1s, "onesf": onesf,
        }
        m.update(w_shared)
        in_maps.append(m)
    return in_maps


def run(inputs, trace=False):
    from concourse.bass_utils import run_bass_kernel_spmd
    nc = _get_program()
    in_maps = _make_in_maps(inputs)
    res = run_bass_kernel_spmd(nc, in_maps, list(range(N_CORES)), trace=trace)
    out = np.zeros((B, D, S), np.float32)
    for c in range(N_CORES):
        b, hh = c // 2, c % 2
        qts = QTS0 if hh == 0 else QTS1
        o = res.results[c]["out"]  # [128, 8, 512]
        o = o.transpose(1, 0, 2).reshape(D, 512)
        for j, qt in enumerate(qts):
            out[b][:, qt * 128:(qt + 1) * 128] = o[:, j * 128:(j + 1) * 128]
    return out, res


def kernel(**inputs):
    out, _ = run(inputs, trace=False)
    return out


# revision 12
# speedup vs baseline: 1.4182x; 1.0059x over previous
"""Trainium2 Bass kernel for a decoder layer (self-attn + cross-attn + MLP,
custom global norm), sharded over 8 NeuronCores as 4 samples x 2 q-sets.

Layout: activations [D, S] (d on partitions); weights as lhsT blocks.
Matmuls in bf16 (fp32 PSUM accumulate); residual stream kept fp32.
Causal skip via stride-2 q-tile assignment (core h owns q-tiles
{6,4,2,0} or {7,5,3,1}, descending) -> identical score-width table on
both cores; masks are per-core data. Softmax denominator comes free from
a ones-augmented V (row 64 of the AV psum); reciprocal is batched
[16,512] and broadcast via a DRAM-roundtrip stride-0 DMA. Norm stats are
fused into the z-producing ops via accum_out; cross-core reduction is a
2-float AllReduce per norm over core pairs.
"""
import sys
sys.path.insert(0, '/opt/trn_rl_repo')
import numpy as np
import ml_dtypes

BF16NP = ml_dtypes.bfloat16

B, D, S, H, DH, DFF = 4, 1024, 1024, 16, 64, 4096
N_CORES = 8
NUDGE = 1e-7
NTOT = float(D * S)
RG = [[0, 1], [2, 3], [4, 5], [6, 7]]
# uniform per-kt score width (q cols 0:n are valid, descending q-tiles)
KTN = [512, 512, 384, 384, 256, 256, 128, 128]
QTS0 = [6, 4, 2, 0]   # local q-tile order, core h==0
QTS1 = [7, 5, 3, 1]   # core h==1


def _split_multi_waits(nc, mybir):
    """walrus codegen allows at most one sync-wait command per instruction;
    move extra waits onto same-engine NoOps inserted just before."""
    n = 0
    for f in nc.m.functions:
        for bb in f.blocks:
            new_insts = []
            for inst in bb.instructions:
                si = getattr(inst, "sync_info", None)
                eng = getattr(inst, "engine", None)
                if si is not None and si.on_wait and len(si.on_wait) > 1 \
                        and eng is not None:
                    waits = list(si.on_wait)
                    for i, w in enumerate(waits[:-1]):
                        nop = mybir.InstNoOp(
                            name=f"{inst.name}-wsplit{i}",
                            engine=eng,
                            sync_info=mybir.SyncInfo(on_wait=[w], on_update=[]),
                            bass_nofuse=True,
                        )
                        new_insts.append(nop)
                        n += 1
                    si.on_wait = [waits[-1]]
                new_insts.append(inst)
            bb.instructions[:] = new_insts
    return n


def build_program():
    import concourse.bass as bass
    import concourse.tile as tile
    from concourse import mybir

    FP32 = mybir.dt.float32
    BF = mybir.dt.bfloat16
    AF = mybir.ActivationFunctionType
    ALU = mybir.AluOpType
    AX = mybir.AxisListType

    nc = bass.Bass("TRN2", target_bir_lowering=False, debug=False,
                   num_devices=N_CORES)

    def din(name, shape, dt=BF):
        return nc.dram_tensor(name, shape, dt, kind="ExternalInput").ap()

    x_d = din("x", [128, 8, 1024])
    xq_d = din("xq", [128, 8, 512])
    xqf_d = din("xqf", [128, 8, 512], FP32)
    emb_d = din("emb", [128, 8, 1024])
    msk_d = din("msk", [128, 8, 128])
    wq_s_d = din("wq_s", [8, 128, 8, 128]); wk_s_d = din("wk_s", [8, 128, 8, 128])
    wv_s_d = din("wv_s", [2, 128, 8, 512]); wo_s_d = din("wo_s", [8, 128, 8, 128])
    wq_c_d = din("wq_c", [8, 128, 8, 128]); wk_c_d = din("wk_c", [8, 128, 8, 128])
    wv_c_d = din("wv_c", [2, 128, 8, 512]); wo_c_d = din("wo_c", [8, 128, 8, 128])
    w1_d = din("w1", [32, 128, 8, 128]); w2_d = din("w2", [8, 128, 32, 128])
    b1_d = din("b1m", [128, 32], FP32)
    b2_d = din("b2m", [128, 8], FP32)
    w1s_d = din("w1s", [128, 32], FP32)
    onesf_d = din("onesf", [128, 1], FP32)
    out_d = nc.dram_tensor("out", [128, 8, 512], FP32,
                           kind="ExternalOutput").ap()

    with tile.TileContext(nc) as tc:
        import contextlib
        ctx = contextlib.ExitStack()
        with ctx:
            persist = ctx.enter_context(tc.tile_pool(name="persist", bufs=1))
            dram = ctx.enter_context(
                tc.tile_pool(name="dram", bufs=1, space="DRAM"))

            xq_sb = persist.tile([128, 8, 512], BF)
            nc.sync.dma_start(out=xq_sb, in_=xq_d)
            xqf_sb = persist.tile([128, 8, 512], FP32)
            msk_sb = persist.tile([128, 8, 128], BF)
            nc.sync.dma_start(out=msk_sb, in_=msk_d)
            b1_sb = persist.tile([128, 32], FP32)
            nc.sync.dma_start(out=b1_sb, in_=b1_d)
            b2_sb = persist.tile([128, 8], FP32)
            nc.sync.dma_start(out=b2_sb, in_=b2_d)
            w1s_sb = persist.tile([128, 32], FP32)
            nc.sync.dma_start(out=w1s_sb, in_=w1s_d)
            onesf_sb = persist.tile([128, 1], FP32)
            nc.sync.dma_start(out=onesf_sb, in_=onesf_d)

            cc_in = [dram.tile([1, 2], FP32, name=f"cc_in{i}", tag=f"cci{i}")
                     for i in range(3)]
            cc_out = [dram.tile([1, 2], FP32, name=f"cc_out{i}", tag=f"cco{i}")
                      for i in range(3)]
            dnr_dr = [dram.tile([16, 512], FP32, name=f"dnr{i}", tag=f"dnr{i}")
                      for i in range(2)]

            def stats_finish(cc_idx, cols_sb, statp):
                """cols [128,16] (z sums in 0:8, sq sums in 8:16) ->
                AllReduce -> (rcol, nbias) columns."""
                st = statp.tile([128, 8], FP32, tag=f"st{cc_idx}",
                                name=f"st{cc_idx}")
                with tc.tile_pool(name=f"sps{cc_idx}", bufs=1,
                                  space="PSUM") as sps:
                    ps = sps.tile([1, 16], FP32, tag="sp")
                    nc.tensor.matmul(ps, onesf_sb, cols_sb,
                                     start=True, stop=True)
                    # sum the two groups of 8 into adjacent scalars
                    nc.vector.tensor_reduce(st[0:1, 6:7], ps[0:1, 0:8],
                                            AX.X, ALU.add)
                    nc.vector.tensor_reduce(st[0:1, 7:8], ps[0:1, 8:16],
                                            AX.X, ALU.add)
                nc.sync.dma_start(out=cc_in[cc_idx], in_=st[0:1, 6:8])
                nc.gpsimd.collective_compute(
                    "AllReduce", ALU.add, replica_groups=RG,
                    ins=[cc_in[cc_idx]], outs=[cc_out[cc_idx]])
                gs = st[:, 4:6]
                bco = cc_out[cc_idx]
                bcast = bass.AP(tensor=bco.tensor, offset=bco.offset,
                                ap=[[0, 128], [1, 2]])
                nc.sync.dma_start(out=gs, in_=bcast)
                s1, s2 = gs[:, 0:1], gs[:, 1:2]
                mean, tmp = st[:, 0:1], st[:, 1:2]
                rcol, nbias = st[:, 2:3], st[:, 3:4]
                nc.vector.tensor_scalar_mul(mean, s1, 1.0 / NTOT)
                nc.vector.tensor_mul(tmp, mean, s1)
                nc.vector.tensor_sub(tmp, s2, tmp)
                nc.scalar.sqrt(tmp, tmp)
                nc.vector.tensor_scalar_add(tmp, tmp, NUDGE)
                nc.vector.reciprocal(rcol, tmp)
                nc.vector.tensor_mul(nbias, mean, rcol)
                nc.vector.tensor_scalar_mul(nbias, nbias, -1.0)
                return rcol, nbias

            kp_ctr = [0]

            def kproj(dst_sb, src_sb, w_dram, wpool, nsblk, copy_eng):
                """dst[do-block] = W^T src, LDW reused across s-blocks."""
                kp_ctr[0] += 1
                with tc.tile_pool(name=f"kp{kp_ctr[0]}", bufs=4,
                                  space="PSUM") as ppool:
                    for do in range(8):
                        wblk = wpool.tile([128, 8, 128], BF, tag="kw")
                        nc.sync.dma_start(out=wblk, in_=w_dram[do])
                        pss = [ppool.tile([128, 512], FP32, tag="pp",
                                          name=f"pp{do}_{i}")
                               for i in range(nsblk)]
                        for di in range(8):
                            for sb_ in range(nsblk):
                                nc.tensor.matmul(
                                    pss[sb_], wblk[:, di, :],
                                    src_sb[:, di, sb_ * 512:(sb_ + 1) * 512],
                                    start=(di == 0), stop=(di == 7))
                        for sb_ in range(nsblk):
                            eng = copy_eng(do, sb_)
                            if eng == "s":
                                nc.scalar.copy(
                                    dst_sb[:, do, sb_ * 512:(sb_ + 1) * 512],
                                    pss[sb_])
                            else:
                                nc.vector.tensor_copy(
                                    dst_sb[:, do, sb_ * 512:(sb_ + 1) * 512],
                                    pss[sb_])

            def vproj(v_sb, src_sb, wv_dram, aph):
                """v_sb [128, 8st, 16h, 65]; col 64 = ones (dn trick)."""
                with (
                    tc.tile_pool(name=f"wv{aph}", bufs=1) as wvp,
                    tc.tile_pool(name=f"vp{aph}", bufs=4,
                                 space="PSUM") as ppool,
                ):
                    wvh = [wvp.tile([128, 8, 512], BF, tag=f"wv{dvb}",
                                    name=f"wvh{dvb}")
                           for dvb in range(2)]
                    for dvb in range(2):
                        nc.sync.dma_start(out=wvh[dvb], in_=wv_dram[dvb])
                    nc.vector.memset(v_sb[:, :, :, 64:65], 1.0)
                    for st_ in range(8):
                        pss = [ppool.tile([128, 512], FP32, tag="vp",
                                          name=f"vp{st_}_{i}")
                               for i in range(2)]
                        for di in range(8):
                            for dvb in range(2):
                                nc.tensor.matmul(
                                    pss[dvb],
                                    src_sb[:, di, st_ * 128:(st_ + 1) * 128],
                                    wvh[dvb][:, di, :],
                                    start=(di == 0), stop=(di == 7))
                        for dvb in range(2):
                            src = pss[dvb].rearrange("p (h m) -> p h m", h=8)
                            nc.vector.tensor_copy(
                                v_sb[:, st_, dvb * 8:(dvb + 1) * 8, 0:64],
                                src)

            def attention(kt_sb, v_sb, qt_sb, wo_dr, use_mask, resid_sb,
                          z_sb, zcols, wpool, dn_idx, aph):
                """Per-head scores/exp/AV with causal width table (self) or
                full table (cross); batched softmax normalize; Wo; residual
                z = Wo-out + resid with fused stat accumulation."""
                ktn = KTN if use_mask else [512] * 8
                aoutf = None
                with (
                    tc.tile_pool(name=f"ao{aph}", bufs=1) as aop,
                    tc.tile_pool(name=f"ep{aph}", bufs=6) as epool,
                    tc.tile_pool(name=f"dn{aph}", bufs=1) as dnpool,
                    tc.tile_pool(name=f"dnb{aph}", bufs=3) as dnbp,
                    tc.tile_pool(name=f"wow{aph}", bufs=1) as wowp,
                ):
                    aoutf = aop.tile([128, 8, 512], FP32, tag="aof")
                    aoutb = aop.tile([128, 8, 512], BF, tag="aob")
                    dnh = [dnpool.tile([8, 512], FP32, tag=f"dn{half}",
                                       name=f"dn{half}") for half in range(2)]
                    wo_w = [wowp.tile([128, 8, 128], BF, tag=f"wo{do}",
                                      name=f"wo{do}") for do in range(8)]
                    for do in range(8):
                        nc.sync.dma_start(out=wo_w[do], in_=wo_dr[do])

                    def norm_half(half):
                        """reciprocal + broadcast + normalize heads of one
                        half (heads 8*half .. 8*half+7 -> hp 4*half..)."""
                        dnp = dnh[half]
                        nc.vector.reciprocal(dnp, dnp)
                        dst = dnr_dr[dn_idx][8 * half:8 * half + 8]
                        nc.sync.dma_start(out=dst, in_=dnp)
                        for hp in range(4 * half, 4 * half + 4):
                            dnb = dnbp.tile([128, 512], FP32, tag="dnb",
                                            name="dnb")
                            for par in range(2):
                                row = dnr_dr[dn_idx][2 * hp + par]
                                bc = bass.AP(tensor=row.tensor,
                                             offset=row.offset,
                                             ap=[[0, 64], [1, 512]])
                                nc.sync.dma_start(
                                    out=dnb[par * 64:(par + 1) * 64, :],
                                    in_=bc)
                            nc.vector.tensor_mul(aoutb[:, hp, :],
                                                 aoutf[:, hp, :], dnb)

                    hctx = __import__("contextlib").ExitStack()
                    scp = hctx.enter_context(
                        tc.tile_pool(name=f"sc{aph}", bufs=2, space="PSUM"))
                    avp = hctx.enter_context(
                        tc.tile_pool(name=f"av{aph}", bufs=2, space="PSUM"))
                    for h in range(H):
                        if h == 8:
                            norm_half(0)
                        off = (h % 2) * 64
                        hp = h // 2
                        e_tiles = []
                        for tt in range(4):
                            n0, n1 = ktn[2 * tt], ktn[2 * tt + 1]
                            sc = scp.tile([128, 2, 512], FP32, tag="sc")
                            for j, n in ((0, n0), (1, n1)):
                                kt = 2 * tt + j
                                nc.tensor.matmul(
                                    sc[:, j, 0:n],
                                    kt_sb[off:off + 64, hp,
                                          kt * 128:(kt + 1) * 128],
                                    qt_sb[off:off + 64, hp, 0:n],
                                    start=True, stop=True,
                                    tile_position=(off, 0))
                            e = epool.tile([128, 2, 512], BF, tag="e")
                            nc.scalar.activation(e[:, :, 0:n0],
                                                 sc[:, :, 0:n0],
                                                 AF.Exp, scale=0.125)
                            if use_mask:
                                w0 = n0 - 128
                                nc.vector.tensor_mul(
                                    e[:, :, w0:n0], e[:, :, w0:n0],
                                    msk_sb[:, 2 * tt:2 * tt + 2, :])
                            e_tiles.append(e)
                        av = avp.tile([128, 512], FP32, tag="av")
                        for kt in range(8):
                            n = ktn[kt]
                            rhs = e_tiles[kt // 2][:, kt % 2, 0:n]
                            nc.tensor.matmul(
                                av[0:65, 0:n],
                                v_sb[:, kt, h, :], rhs,
                                start=(kt == 0), stop=(kt == 7))
                        stage = dnbp.tile([65, 512], FP32, tag="stg",
                                          name=f"stg{aph}")
                        if h % 2 == 0:
                            nc.vector.tensor_copy(stage, av[0:65, :])
                        else:
                            nc.scalar.copy(stage, av[0:65, :])
                        nc.sync.dma_start(out=dnh[h // 8][h % 8:h % 8 + 1, :],
                                          in_=stage[64:65, :])
                        nc.sync.dma_start(out=aoutf[off:off + 64, hp, :],
                                          in_=stage[0:64, :])
                    hctx.close()
                    norm_half(1)
                    with tc.tile_pool(name=f"wops{aph}", bufs=1,
                                      space="PSUM") as wops:
                        pss = [wops.tile([128, 512], FP32, tag=f"wo{do}",
                                         name=f"wops{do}")
                               for do in range(8)]
                        for di in range(8):
                            for do in range(8):
                                nc.tensor.matmul(pss[do], wo_w[do][:, di, :],
                                                 aoutb[:, di, :],
                                                 start=(di == 0),
                                                 stop=(di == 7))
                        for do in range(8):
                            nc.vector.scalar_tensor_tensor(
                                z_sb[:, do, :], pss[do], 1.0,
                                resid_sb[:, do, :],
                                ALU.mult, ALU.add,
                                accum_out=zcols[:, do:do + 1])
                            nc.scalar.activation(
                                sq_sc[:, do % 2, :], z_sb[:, do, :],
                                AF.Square, accum_out=zcols[:, 8 + do:9 + do])

            # scratch for Square outputs (values unused, accum_out is used)
            sq_sc = persist.tile([128, 2, 512], FP32)

            # ================= self attention =================
            statp = ctx.enter_context(tc.tile_pool(name="stats", bufs=1))
            colsp = ctx.enter_context(tc.tile_pool(name="cols", bufs=1))
            z1_sb = persist.tile([128, 8, 512], FP32)
            zcols1 = colsp.tile([128, 16], FP32, tag="zc1", name="zc1")
            with tc.tile_pool(name="kvS", bufs=1) as kvS:
                ktS = kvS.tile([128, 8, 1024], BF, tag="kt")
                vS = kvS.tile([128, 8, 16, 65], BF, tag="v")
                qtS = kvS.tile([128, 8, 512], BF, tag="qt")
                with tc.tile_pool(name="srcS", bufs=1) as srcS, \
                     tc.tile_pool(name="wstrS", bufs=3) as wpoolS:
                    x_sb = srcS.tile([128, 8, 1024], BF, tag="x")
                    nc.sync.dma_start(out=x_sb, in_=x_d)
                    nc.sync.dma_start(out=xqf_sb, in_=xqf_d)
                    kproj(ktS, x_sb, wk_s_d, wpoolS, 2,
                          lambda do, sb_: "s" if sb_ == 0 else "v")
                    vproj(vS, x_sb, wv_s_d, "s")
                    kproj(qtS, xq_sb, wq_s_d, wpoolS, 1, lambda do, sb_: "s")
                attention(ktS, vS, qtS, wo_s_d, True, xqf_sb,
                          z1_sb, zcols1, None, 0, "s")
            rc1, nb1 = stats_finish(0, zcols1, statp)

            # ============= cross attention =============
            resid2 = ctx.enter_context(tc.tile_pool(name="resid2", bufs=1))
            z2_sb = resid2.tile([128, 8, 512], FP32, tag="z2", name="z2_sb")
            zcols2 = colsp.tile([128, 16], FP32, tag="zc2", name="zc2")
            pa_sb = resid2.tile([128, 8, 512], BF, tag="pa", name="pa_sb")
            paf_sb = resid2.tile([128, 8, 512], FP32, tag="paf",
                                 name="paf_sb")
            with tc.tile_pool(name="kvC", bufs=1) as kvC:
                ktC = kvC.tile([128, 8, 1024], BF, tag="kt")
                vC = kvC.tile([128, 8, 16, 65], BF, tag="v")
                qtC = kvC.tile([128, 8, 512], BF, tag="qt")
                with tc.tile_pool(name="srcC", bufs=1) as srcC, \
                     tc.tile_pool(name="wstrC", bufs=3) as wpoolC:
                    e_sb = srcC.tile([128, 8, 1024], BF, tag="emb")
                    nc.sync.dma_start(out=e_sb, in_=emb_d)
                    # K/V of emb overlap the norm1 AllReduce
                    kproj(ktC, e_sb, wk_c_d, wpoolC, 2,
                          lambda do, sb_: "s" if sb_ == 0 else "v")
                    vproj(vC, e_sb, wv_c_d, "c")
                    for di in range(8):
                        nc.scalar.activation(pa_sb[:, di, :], z1_sb[:, di, :],
                                             AF.Identity, bias=nb1, scale=rc1)
                        nc.vector.tensor_scalar(paf_sb[:, di, :],
                                                z1_sb[:, di, :], rc1, nb1,
                                                ALU.mult, ALU.add)
                    kproj(qtC, pa_sb, wq_c_d, wpoolC, 1, lambda do, sb_: "s")
                attention(ktC, vC, qtC, wo_c_d, False, paf_sb,
                          z2_sb, zcols2, None, 1, "c")
            rc2, nb2 = stats_finish(1, zcols2, statp)

            # ================= MLP =================
            zcols3 = colsp.tile([128, 16], FP32, tag="zc3", name="zc3")
            with (
                tc.tile_pool(name="mlp", bufs=1) as mlp,
                tc.tile_pool(name="w1str", bufs=3) as w1str,
                tc.tile_pool(name="w2str", bufs=2) as w2str,
            ):
                # M = W1^T z2 (pre-norm; overlaps the norm2 AllReduce)
                z2b_sb = mlp.tile([128, 8, 512], BF, tag="z2b")
                for di in range(8):
                    nc.scalar.copy(z2b_sb[:, di, :], z2_sb[:, di, :])
                m_sb = mlp.tile([128, 32, 512], BF, tag="m")
                with tc.tile_pool(name="m1ps", bufs=4, space="PSUM") as m1ps:
                    for f in range(32):
                        wblk = w1str.tile([128, 8, 128], BF, tag="w1")
                        nc.sync.dma_start(out=wblk, in_=w1_d[f])
                        ps = m1ps.tile([128, 512], FP32, tag="m1")
                        for di in range(8):
                            nc.tensor.matmul(ps, wblk[:, di, :],
                                             z2b_sb[:, di, :],
                                             start=(di == 0), stop=(di == 7))
                        nc.vector.tensor_copy(m_sb[:, f, :], ps)
                # h1 = relu(rc2*M + (nb2*w1s + b1)); pe = rc2*z2 + nb2
                biasf = mlp.tile([128, 32], FP32, tag="biasf")
                nc.vector.tensor_scalar(biasf, w1s_sb, nb2, None, ALU.mult)
                nc.vector.tensor_add(biasf, biasf, b1_sb)
                pe_sb = mlp.tile([128, 8, 512], FP32, tag="pe")
                for di in range(8):
                    nc.vector.tensor_scalar(pe_sb[:, di, :], z2_sb[:, di, :],
                                            rc2, nb2, ALU.mult, ALU.add)
                h1_sb = m_sb
                for f in range(32):
                    nc.scalar.activation(h1_sb[:, f, :], m_sb[:, f, :],
                                         AF.Relu, bias=biasf[:, f:f + 1],
                                         scale=rc2)
                z3_sb = mlp.tile([128, 8, 512], FP32, tag="z3")
                with tc.tile_pool(name="m2ps", bufs=3, space="PSUM") as m2ps:
                    for do in range(8):
                        w2blk = w2str.tile([128, 32, 128], BF, tag="w2")
                        nc.sync.dma_start(out=w2blk, in_=w2_d[do])
                        ps = m2ps.tile([128, 512], FP32, tag="m2")
                        for ff in range(32):
                            nc.tensor.matmul(ps, w2blk[:, ff, :],
                                             h1_sb[:, ff, :],
                                             start=(ff == 0), stop=(ff == 31))
                        nc.vector.scalar_tensor_tensor(
                            z3_sb[:, do, :], ps, b2_sb[:, do:do + 1],
                            pe_sb[:, do, :], ALU.add, ALU.add,
                            accum_out=zcols3[:, do:do + 1])
                        nc.scalar.activation(
                            sq_sc[:, do % 2, :], z3_sb[:, do, :],
                            AF.Square, accum_out=zcols3[:, 8 + do:9 + do])
                rc3, nb3 = stats_finish(2, zcols3, statp)
                with tc.tile_pool(name="outp", bufs=4) as outp:
                    for do in range(8):
                        ot = outp.tile([128, 512], FP32, tag="ot")
                        nc.scalar.activation(ot, z3_sb[:, do, :],
                                             AF.Identity, bias=nb3, scale=rc3)
                        nc.sync.dma_start(out=out_d[:, do, :], in_=ot)

    from concourse import mybir as _mb
    _split_multi_waits(nc, _mb)
    return nc


_CACHE = {}


def _get_program():
    if "nc" not in _CACHE:
        _CACHE["nc"] = build_program()
    return _CACHE["nc"]


def _bf(x):
    return np.ascontiguousarray(np.asarray(x, np.float32)).astype(BF16NP)


def _blk(w, nblk, blk):
    """[K, N] -> [nblk, 128, K//128, blk] contiguous per-column-block."""
    K = w.shape[0]
    return np.ascontiguousarray(
        _bf(w).reshape(K // 128, 128, nblk, blk).transpose(2, 1, 0, 3))


def _dmaj(a):
    """[D, S] -> [128, 8, S] with d = t*128 + p."""
    return np.ascontiguousarray(
        np.asarray(a).reshape(8, 128, a.shape[1]).transpose(1, 0, 2))


def _make_in_maps(inputs):
    w_shared = {}
    for k in ("Wq_s", "Wk_s", "Wo_s", "Wq_c", "Wk_c", "Wo_c"):
        w_shared[k.lower()] = _blk(inputs[k], 8, 128)
    for k in ("Wv_s", "Wv_c"):
        w_shared[k.lower()] = _blk(inputs[k], 2, 512)
    w_shared["w1"] = _blk(inputs["W1"], 32, 128)
    w_shared["w2"] = _blk(inputs["W2"], 8, 128)
    b1m = np.ascontiguousarray(
        np.asarray(inputs["b1"], np.float32).reshape(32, 128).T)
    b2m = np.ascontiguousarray(
        np.asarray(inputs["b2"], np.float32).reshape(8, 128).T)
    w1s = np.ascontiguousarray(
        _bf(inputs["W1"]).astype(np.float64).sum(axis=0).astype(
            np.float32).reshape(32, 128).T)
    onesf = np.ones((128, 1), np.float32)
    tri = (np.arange(128)[:, None] <= np.arange(128)[None, :])

    in_maps = []
    for c in range(N_CORES):
        b, hh = c // 2, c % 2
        qts = QTS0 if hh == 0 else QTS1
        x_f = np.asarray(inputs["other_inputs"][b], np.float32)
        xq_f = np.concatenate([x_f[:, qt * 128:(qt + 1) * 128] for qt in qts],
                              axis=1)
        msk = np.zeros((128, 8, 128), np.float32)
        for kt in range(8):
            if hh == 0:
                msk[:, kt] = tri if kt % 2 == 0 else 0.0
            else:
                msk[:, kt] = 1.0 if kt % 2 == 0 else tri
        m = {
            "x": _bf(_dmaj(x_f)),
            "xq": _bf(_dmaj(xq_f)),
            "xqf": _dmaj(xq_f).astype(np.float32),
            "emb": _bf(_dmaj(np.asarray(inputs["embedding"][b], np.float32))),
            "msk": _bf(msk),
            "b1m": b1m, "b2m": b2m, "w1s": w1s, "onesf": onesf,
        }
        m.update(w_shared)
        in_maps.append(m)
    return in_maps


def run(inputs, trace=False):
    from concourse.bass_utils import run_bass_kernel_spmd
    nc = _get_program()
    in_maps = _make_in_maps(inputs)
    res = run_bass_kernel_spmd(nc, in_maps, list(range(N_CORES)), trace=trace)
    out = np.zeros((B, D, S), np.float32)
    for c in range(N_CORES):
        b, hh = c // 2, c % 2
        qts = QTS0 if hh == 0 else QTS1
        o = res.results[c]["out"]  # [128, 8, 512]
        o = o.transpose(1, 0, 2).reshape(D, 512)
        for j, qt in enumerate(qts):
            out[b][:, qt * 128:(qt + 1) * 128] = o[:, j * 128:(j + 1) * 128]
    return out, res


def kernel(**inputs):
    out, _ = run(inputs, trace=False)
    return out
